# revision 2
# baseline (speedup 1.0000x reference)
"""Trainium2 Bass kernel for the ConexaoRegional locally-connected layer.

Computation:  z[b, n, d, s] = sum_{h,w} region_n(x[b])[h, w] * pesos[n, d, s, h, w]
  x:     [32, 1, 256, 256] f32   -> host-cast bf16
  pesos: [4096, 16, 16, 4, 4] f32 -> host-cast bf16
  out:   [32, 4096, 16, 16] f32  <- device computes f32 PSUM, stores bf16,
                                    host upcasts (rel_err ~2.7e-3)

Sharding: tensor-parallel over regions (N) across 8 cores; core c owns
regions [512c, 512c+512) (x pixel rows [32c, 32c+32)). Per-core HBM
traffic 13MB (x 1MB + w 4MB + o 8MB) vs 26MB for the f32 version.

Key structure — fused-parity M=64 matmuls (256 PE instructions/rep):

The K=32 padded-parity x layout zeroes the wrong-parity half of the
contraction, so one matmul whose stationary spans BOTH groups of a pair
(M = (u, b) = 64, a strided 2-D free AP) computes both parities against
the same K=32 W columns:
  out[(u, b), ds] = sum_k x_pad[k', (u, b)] * w[k', ds]
with x_pad zero outside parity-u rows. PE tiles: rows 32i, cols {0, 64}
(hh = hg%2); the two hgH = hg//2 values serialize per tile.

PSUM layout per gp: partition = 64*hh + 32*u + b, free = 512*i + 256*hgH + ds.
o[p, 128, 4096]: free = 2048*(gp%2) + psum free.

Per-core HBM traffic: x 1MB + w 4MB + o 8MB = 13MB bf16/f32-free.
"""

import numpy as np
import ml_dtypes

N_CORES = 8
B = 32
N_REG = 4096
DS = 256
K = 16
RPC = N_REG // N_CORES  # 512
NG = 32                 # groups per core
NP = NG // 2            # group pairs
BF16 = ml_dtypes.bfloat16

_CACHE = {}


def _build_nc(reps=1, dyn_reps=1):
    import contextlib

    import concourse.bacc as bacc
    import concourse.mybir as mybir
    import concourse.tile as tile

    F32 = mybir.dt.float32
    BF = mybir.dt.bfloat16
    nc = bacc.Bacc("TRN2", target_bir_lowering=False, debug=False)
    x_d = nc.dram_tensor("x", [128, NP, 256], BF, kind="ExternalInput")
    w_d = nc.dram_tensor("w", [128, NP, 1024], BF, kind="ExternalInput")
    o_d = nc.dram_tensor("o", [NP // 2, 128, 4096], BF, kind="ExternalOutput")

    with tile.TileContext(nc) as tc:
        with (
            tc.tile_pool(name="xsb", bufs=2) as xp,
            tc.tile_pool(name="wsb", bufs=2) as wp,
            tc.tile_pool(name="ostage", bufs=4) as op,
            tc.tile_pool(name="pso", bufs=2, space="PSUM") as psop,
        ):
            wflat = w_d.ap().rearrange("p gp f -> p (gp f)")

            loop_cm = (
                tc.For_i(0, dyn_reps, 1)
                if dyn_reps > 1
                else contextlib.nullcontext()
            )
            with loop_cm:
                for rep in range(reps):
                    _one_rep(nc, x_d, wflat, o_d, xp, wp, op, psop, F32, BF)

    nc.compile()
    return nc


def _one_rep(nc, x_d, wflat, o_d, xp, wp, op, psop, F32, BF):
    xsb = xp.tile([128, NG * 128], BF)
    wsb = wp.tile([128, NP * 1024], BF)
    # x on SWDGE (gpsimd); all W chunks on the sync HWDGE ring, so stores
    # (scalar/gpsimd) never queue ahead of W loads.
    nc.gpsimd.dma_start(out=xsb[:], in_=x_d.ap().rearrange("p g c -> p (g c)"))
    for q in range(4):
        nc.sync.dma_start(
            out=wsb[:, 4096 * q : 4096 * (q + 1)],
            in_=wflat[:, 4096 * q : 4096 * (q + 1)],
        )
    ostage = None
    for gp in range(NP):
        psum_o = psop.tile([128, 2048], F32)
        for i in range(4):
            for hg in range(4):
                hh, hgH = hg % 2, hg // 2
                nc.tensor.matmul(
                    psum_o[
                        64 * hh : 64 * hh + 64,
                        512 * i + 256 * hgH : 512 * i + 256 * hgH + 256,
                    ],
                    xsb[
                        32 * i : 32 * i + 32,
                        256 * gp + 64 * hg : 256 * gp + 64 * hg + 64,
                    ],
                    wsb[
                        32 * i : 32 * i + 32,
                        1024 * gp + 256 * hg : 1024 * gp + 256 * hg + 256,
                    ],
                    start=True,
                    stop=True,
                    tile_position=(32 * i, 64 * hh),
                )
        if gp % 2 == 0:
            ostage = op.tile([128, 4096], BF)
        off = 2048 * (gp % 2)
        # PSUM -> SBUF with f32->bf16 cast, split across DVE and ACT.
        nc.vector.tensor_copy(
            out=ostage[:, off : off + 1024], in_=psum_o[:, :1024]
        )
        nc.scalar.copy(out=ostage[:, off + 1024 : off + 2048], in_=psum_o[:, 1024:])
        if gp % 2 == 1:
            p = gp // 2
            store_eng = (nc.scalar, nc.gpsimd)[p % 2]
            store_eng.dma_start(out=o_d.ap()[p], in_=ostage[:])


def _prep_in_maps(x, pesos):
    """Full f32 inputs -> 8 per-core bf16 input dicts (host-side prep)."""
    x = np.asarray(x, dtype=np.float32)
    pesos = np.asarray(pesos, dtype=np.float32)
    # pesos [n, d, s, h, w] -> [n, k=(h*4+w), ds=(d*16+s)]
    pesos_t = np.ascontiguousarray(pesos.transpose(0, 3, 4, 1, 2)).reshape(
        N_REG, K, DS
    )
    in_maps = []
    for c in range(N_CORES):
        # x regions for this core: [b, n_local, k]
        x_c = x[:, 0, 32 * c : 32 * c + 32, :]
        xr = (
            x_c.reshape(B, 8, 4, 64, 4)
            .transpose(0, 1, 3, 2, 4)
            .reshape(B, RPC, K)
        )
        # xt[32i+16u+k, 256gp+64hg+32u'+b] = xr[b, 16(2gp+u')+4i+hg, k] iff u'==u
        a = xr.reshape(B, NP, 2, 4, 4, K)              # b, gp, u', i, hg, k
        a_t = a.transpose(3, 2, 5, 1, 4, 0).astype(BF16)  # i, u', k, gp, hg, b
        xt = np.zeros((4, 2, K, NP, 4, 2, B), dtype=BF16)  # i,u,k,gp,hg,u',b
        xt[:, 0, :, :, :, 0] = a_t[:, 0]
        xt[:, 1, :, :, :, 1] = a_t[:, 1]
        xt = xt.reshape(128, NP, 256)

        # w[32i + 16u + k, gp, 256hg + ds] = pesos_t[512c + 16(2gp+u)+4i+hg, k, ds]
        wc = pesos_t[512 * c : 512 * (c + 1)].reshape(NP, 2, 4, 4, K, DS)
        w_arr = np.ascontiguousarray(
            wc.transpose(2, 1, 4, 0, 3, 5).astype(BF16)  # i, u, k, gp, hg, ds
        ).reshape(128, NP, 1024)

        in_maps.append({"x": np.ascontiguousarray(xt), "w": w_arr})
    return in_maps


def _unshard(results):
    """Per-core bf16 outputs -> full f32 [B, N, 16, 16]."""
    out = np.empty((B, N_REG, DS), dtype=BF16)
    for c, res in enumerate(results):
        # o: [p, (64hh+32u+b), (q, i, hgH, ds)] with gp = 2p + q, hg = 2hgH + hh
        o_c = res["o"].reshape(NP // 2, 2, 2, B, 2, 4, 2, DS)
        # axes: p, hh, u, b, q, i, hgH, ds
        o_t = o_c.transpose(3, 0, 4, 2, 5, 6, 1, 7)  # b, p, q, u, i, hgH, hh, ds
        out[:, 512 * c : 512 * (c + 1), :] = o_t.reshape(B, RPC, DS)
    return out.reshape(B, N_REG, 16, 16).astype(np.float32)


def kernel(x, pesos):
    from concourse.bass_utils import run_bass_kernel_spmd

    if "nc" not in _CACHE:
        _CACHE["nc"] = _build_nc()
    nc = _CACHE["nc"]
    in_maps = _prep_in_maps(x, pesos)
    res = run_bass_kernel_spmd(nc, in_maps, core_ids=list(range(N_CORES)))
    return _unshard(res.results)


# revision 3
# speedup vs baseline: 1.2995x; 1.2995x over previous
"""Trainium2 Bass kernel for the ConexaoRegional locally-connected layer.

Computation:  z[b, n, d, s] = sum_{h,w} region_n(x[b])[h, w] * pesos[n, d, s, h, w]
  x:     [32, 1, 256, 256] f32   -> host-cast bf16
  pesos: [4096, 16, 16, 4, 4] f32 -> host-cast bf16
  out:   [32, 4096, 16, 16] f32  <- device computes f32 PSUM, stores bf16,
                                    host upcasts (rel_err ~2.7e-3, gate 2e-2)

Sharding: tensor-parallel over regions (N) across 8 cores; core c owns
regions [512c, 512c+512) (x pixel rows [32c, 32c+32)). Per-core HBM
traffic 13MB (x 1MB + w 4MB + o 8MB) vs 26MB for the f32 version.

Key structure — fused-parity M=64 matmuls (256 PE instructions/rep) and
half-size PSUM tiles for early bank release:

The K=32 padded-parity x layout zeroes the wrong-parity half of the
contraction, so one matmul whose stationary spans BOTH groups of a pair
(M = (u, b) = 64, a strided 2-D free AP) computes both parities against
the same K=32 W columns:
  out[(u, b), ds] = sum_k x_pad[k', (u, b)] * w[k', ds]
with x_pad zero outside parity-u rows. PE tiles: rows 32i, cols {0, 64}
(hh = hg%2); the two hgH = hg//2 values serialize per tile.

PSUM layout per gp: partition = 64*hh + 32*u + b, free = 512*i + 256*hgH + ds.
o[p, 128, 4096]: free = 2048*(gp%2) + psum free.

Per-core HBM traffic: x 1MB + w 4MB + o 8MB = 13MB bf16/f32-free.
"""

import numpy as np
import ml_dtypes

N_CORES = 8
B = 32
N_REG = 4096
DS = 256
K = 16
RPC = N_REG // N_CORES  # 512
NG = 32                 # groups per core
NP = NG // 2            # group pairs
BF16 = ml_dtypes.bfloat16

_CACHE = {}


def _build_nc(reps=1, dyn_reps=1):
    import contextlib

    import concourse.bacc as bacc
    import concourse.mybir as mybir
    import concourse.tile as tile

    F32 = mybir.dt.float32
    BF = mybir.dt.bfloat16
    nc = bacc.Bacc("TRN2", target_bir_lowering=False, debug=False)
    x_d = nc.dram_tensor("x", [128, NP, 256], BF, kind="ExternalInput")
    w_d = nc.dram_tensor("w", [128, NP, 1024], BF, kind="ExternalInput")
    o_d = nc.dram_tensor("o", [NP // 2, 128, 4096], BF, kind="ExternalOutput")

    with tile.TileContext(nc) as tc:
        with (
            tc.tile_pool(name="xsb", bufs=2) as xp,
            tc.tile_pool(name="wsb", bufs=2) as wp,
            tc.tile_pool(name="ostage", bufs=4) as op,
            tc.tile_pool(name="pso", bufs=2, space="PSUM") as psop,
        ):
            wflat = w_d.ap().rearrange("p gp f -> p (gp f)")

            loop_cm = (
                tc.For_i(0, dyn_reps, 1)
                if dyn_reps > 1
                else contextlib.nullcontext()
            )
            with loop_cm:
                for rep in range(reps):
                    _one_rep(nc, x_d, wflat, o_d, xp, wp, op, psop, F32, BF)

    nc.compile()
    return nc


def _one_rep(nc, x_d, wflat, o_d, xp, wp, op, psop, F32, BF):
    xsb = xp.tile([128, NG * 128], BF)
    wsb = wp.tile([128, NP * 1024], BF)
    # x on SWDGE (gpsimd); all W chunks on the sync HWDGE ring, so stores
    # (scalar/gpsimd) never queue ahead of W loads.
    nc.gpsimd.dma_start(out=xsb[:], in_=x_d.ap().rearrange("p g c -> p (g c)"))
    for q in range(4):
        nc.sync.dma_start(
            out=wsb[:, 4096 * q : 4096 * (q + 1)],
            in_=wflat[:, 4096 * q : 4096 * (q + 1)],
        )
    ostage = None
    for gp in range(NP):
        # Two half-size PSUM tiles per group pair (i<2 / i>=2): each bank
        # pair is released back to the PE as soon as its own evac completes,
        # instead of waiting for the full 2048-col evacuation. Measured -7us.
        ps_a = psop.tile([128, 1024], F32)
        ps_b = psop.tile([128, 1024], F32)
        for i in range(4):
            for hg in range(4):
                hh, hgH = hg % 2, hg // 2
                nc.tensor.matmul(
                    (ps_a, ps_b)[i // 2][
                        64 * hh : 64 * hh + 64,
                        512 * (i % 2) + 256 * hgH : 512 * (i % 2)
                        + 256 * hgH
                        + 256,
                    ],
                    xsb[
                        32 * i : 32 * i + 32,
                        256 * gp + 64 * hg : 256 * gp + 64 * hg + 64,
                    ],
                    wsb[
                        32 * i : 32 * i + 32,
                        1024 * gp + 256 * hg : 1024 * gp + 256 * hg + 256,
                    ],
                    start=True,
                    stop=True,
                    tile_position=(32 * i, 64 * hh),
                )
        if gp % 2 == 0:
            ostage = op.tile([128, 4096], BF)
        off = 2048 * (gp % 2)
        # PSUM -> SBUF with f32->bf16 cast, split across DVE and ACT.
        nc.vector.tensor_copy(out=ostage[:, off : off + 1024], in_=ps_a[:])
        nc.scalar.copy(out=ostage[:, off + 1024 : off + 2048], in_=ps_b[:])
        if gp % 2 == 1:
            p = gp // 2
            store_eng = (nc.scalar, nc.gpsimd)[p % 2]
            store_eng.dma_start(out=o_d.ap()[p], in_=ostage[:])


def _prep_in_maps(x, pesos):
    """Full f32 inputs -> 8 per-core bf16 input dicts (host-side prep)."""
    x = np.asarray(x, dtype=np.float32)
    pesos = np.asarray(pesos, dtype=np.float32)
    # pesos [n, d, s, h, w] -> [n, k=(h*4+w), ds=(d*16+s)]
    pesos_t = np.ascontiguousarray(pesos.transpose(0, 3, 4, 1, 2)).reshape(
        N_REG, K, DS
    )
    in_maps = []
    for c in range(N_CORES):
        # x regions for this core: [b, n_local, k]
        x_c = x[:, 0, 32 * c : 32 * c + 32, :]
        xr = (
            x_c.reshape(B, 8, 4, 64, 4)
            .transpose(0, 1, 3, 2, 4)
            .reshape(B, RPC, K)
        )
        # xt[32i+16u+k, 256gp+64hg+32u'+b] = xr[b, 16(2gp+u')+4i+hg, k] iff u'==u
        a = xr.reshape(B, NP, 2, 4, 4, K)              # b, gp, u', i, hg, k
        a_t = a.transpose(3, 2, 5, 1, 4, 0).astype(BF16)  # i, u', k, gp, hg, b
        xt = np.zeros((4, 2, K, NP, 4, 2, B), dtype=BF16)  # i,u,k,gp,hg,u',b
        xt[:, 0, :, :, :, 0] = a_t[:, 0]
        xt[:, 1, :, :, :, 1] = a_t[:, 1]
        xt = xt.reshape(128, NP, 256)

        # w[32i + 16u + k, gp, 256hg + ds] = pesos_t[512c + 16(2gp+u)+4i+hg, k, ds]
        wc = pesos_t[512 * c : 512 * (c + 1)].reshape(NP, 2, 4, 4, K, DS)
        w_arr = np.ascontiguousarray(
            wc.transpose(2, 1, 4, 0, 3, 5).astype(BF16)  # i, u, k, gp, hg, ds
        ).reshape(128, NP, 1024)

        in_maps.append({"x": np.ascontiguousarray(xt), "w": w_arr})
    return in_maps


def _unshard(results):
    """Per-core bf16 outputs -> full f32 [B, N, 16, 16]."""
    out = np.empty((B, N_REG, DS), dtype=BF16)
    for c, res in enumerate(results):
        # o: [p, (64hh+32u+b), (q, i, hgH, ds)] with gp = 2p + q, hg = 2hgH + hh
        o_c = res["o"].reshape(NP // 2, 2, 2, B, 2, 4, 2, DS)
        # axes: p, hh, u, b, q, i, hgH, ds
        o_t = o_c.transpose(3, 0, 4, 2, 5, 6, 1, 7)  # b, p, q, u, i, hgH, hh, ds
        out[:, 512 * c : 512 * (c + 1), :] = o_t.reshape(B, RPC, DS)
    return out.reshape(B, N_REG, 16, 16).astype(np.float32)


def kernel(x, pesos):
    from concourse.bass_utils import run_bass_kernel_spmd

    if "nc" not in _CACHE:
        _CACHE["nc"] = _build_nc()
    nc = _CACHE["nc"]
    in_maps = _prep_in_maps(x, pesos)
    res = run_bass_kernel_spmd(nc, in_maps, core_ids=list(range(N_CORES)))
    return _unshard(res.results)
